# revision 1
# baseline (speedup 1.0000x reference)
"""Trainium2 Bass kernel for nn_AutoregressiveSelfAttention.

Sharding (8 cores): batch (2-way) x head-group (4-way tensor parallel).
Core c: batch c//4, heads [4*(c%4), 4*(c%4)+4).
Per-core: fp16 matmuls throughout (QKV proj, scores, P@V, out-proj),
fp32 softmax statistics, fp32 partial output; host sums the 4 head-group
partials per batch (the row-parallel all-reduce) and transposes back.

Softmax without transposes: pass1 computes scores [sq, sk] only to get the
causal row-max M (fused mask+max on DVE); pass2 recomputes scores
transposed with the max-subtraction folded in as a rank-1 matmul term
([kT;1].T @ [qT;-M]), exps on ACT into fp16, and the ctx matmul against
[v|1] accumulates both ctx and the softmax denominator in one PSUM tile.
Emission is interleaved per (seq-chunk, head-pair) so the DVE-bound pass1
and ACT-bound pass2 overlap.
"""
import sys
sys.path.insert(0, "/opt/trn_rl_repo")

import math
import numpy as np

B, S, E, H, D = 2, 2048, 1024, 16, 64
NCORES = 8
HG = 4                  # head-group shards
HPC = H // HG           # 4 heads per core
OC = HPC * D            # 256 per-core projection width
P = 128
NK = E // P             # 8 contraction tiles for projections
NT = S // P             # 16 seq tiles of 128
NJ = S // 512           # 4 seq chunks of 512

_CACHE = {}


def _build():
    import concourse.bacc as bacc
    import concourse.mybir as mybir
    import concourse.tile as tile
    from concourse.masks import make_identity, make_causal_mask

    dt = mybir.dt
    f32, f16 = dt.float32, dt.float16
    AX = mybir.AxisListType.X
    ALU = mybir.AluOpType

    nc = bacc.Bacc(None, target_bir_lowering=False, debug=False)
    with tile.TileContext(nc) as tc:
        with tc.tile_pool(name="dram", bufs=1, space="DRAM") as dram, \
             tc.tile_pool(name="persist", bufs=1) as pers, \
             tc.tile_pool(name="stream", bufs=4) as strm, \
             tc.tile_pool(name="tmp", bufs=4) as tmp, \
             tc.tile_pool(name="ps", bufs=1, space="PSUM") as ps:

            # ---- DRAM I/O ----
            xt = dram.tile([E, S], f16, kind="ExternalInput", name="xt", uniquify=False)
            wq = dram.tile([E, OC], f16, kind="ExternalInput", name="wq", uniquify=False)
            wk = dram.tile([E, OC], f16, kind="ExternalInput", name="wk", uniquify=False)
            wv = dram.tile([E, OC], f16, kind="ExternalInput", name="wv", uniquify=False)
            wo = dram.tile([OC, E], f16, kind="ExternalInput", name="wo", uniquify=False)
            outT = dram.tile([E, S], f32, kind="ExternalOutput", name="outT", uniquify=False)

            # ---- persistent SBUF ----
            xt_sb = pers.tile([P, NK, S], f16)
            wq_sb = pers.tile([P, NK, OC], f16)
            wk_sb = pers.tile([P, NK, OC], f16)
            wv_sb = pers.tile([P, NK, OC], f16)
            wo_sb = pers.tile([P, 2, E], f16)
            qp_sb = pers.tile([P, 2, S], f16)       # qT, head-pair stacked
            kp_sb = pers.tile([P, 2, S], f16)       # kT, head-pair stacked
            qaug = pers.tile([65, HPC, S], f16)     # [qT_h ; -M_h] per head
            kaug = pers.tile([65, HPC, S], f16)     # [kT_h ; ones] per head
            vv0 = pers.tile([P, NT, 2, 65], f16)    # heads 0,2: [v(0:64) | ones(64)]
            vv1 = pers.tile([P, NT, 2, P], f16)     # heads 1,3: [ones | 0*63 | v(64:128)]
            ctxn = pers.tile([P, 2, S], f16)        # normalized ctx, pair stacked
            m2 = pers.tile([P, 2, 32], f32)         # rowmax cols per pair (hh*16+t)
            ident = pers.tile([P, P], f32)
            ident16 = pers.tile([P, P], f16)
            cmask16 = pers.tile([P, P], f16)        # 0 / -30000 causal block

            # ---- input DMAs ----
            xt_v = xt[:].rearrange("(k p) s -> k p s", p=P)
            wq_v = wq[:].rearrange("(k p) o -> k p o", p=P)
            wk_v = wk[:].rearrange("(k p) o -> k p o", p=P)
            wv_v = wv[:].rearrange("(k p) o -> k p o", p=P)
            wo_v = wo[:].rearrange("(k p) e -> k p e", p=P)
            outT_v = outT[:].rearrange("(o p) s -> o p s", p=P)
            for k in range(NK):
                nc.sync.dma_start(out=xt_sb[:, k, :], in_=xt_v[k])
                nc.sync.dma_start(out=wq_sb[:, k, :], in_=wq_v[k])
                nc.sync.dma_start(out=wk_sb[:, k, :], in_=wk_v[k])
                nc.sync.dma_start(out=wv_sb[:, k, :], in_=wv_v[k])
            for kt in range(2):
                nc.sync.dma_start(out=wo_sb[:, kt, :], in_=wo_v[kt])

            # ---- constants ----
            make_identity(nc, ident[:, :])
            make_identity(nc, ident16[:, :])
            make_causal_mask(nc, cmask16[:, :], mask_val=-30000.0)
            nc.gpsimd.memset(kaug[64:65, :, :], 1.0)
            nc.gpsimd.memset(vv0[:, :, :, 64:65], 1.0)
            nc.gpsimd.memset(vv1[:, :, :, 0:1], 1.0)
            nc.gpsimd.memset(vv1[:, :, :, 1:64], 0.0)

            # ---- q/k projections (transposed layout, pair-stacked) ----
            for dst, w_sb in ((qp_sb, wq_sb), (kp_sb, wk_sb)):
                for ot in range(2):
                    for j in range(NJ):
                        pp = ps.tile([P, 512], f32, tag="proj", bufs=2)
                        for k in range(NK):
                            nc.tensor.matmul(
                                pp[:, :],
                                w_sb[:, k, 128 * ot:128 * ot + 128],
                                xt_sb[:, k, 512 * j:512 * j + 512],
                                start=(k == 0), stop=(k == NK - 1))
                        nc.vector.tensor_copy(dst[:, ot, 512 * j:512 * j + 512],
                                              pp[:, :])

            # ---- augmented qT/kT copies (partition shift -> DMA) ----
            def emit_aug(j):
                for h in range(HPC):
                    pr, hh = divmod(h, 2)
                    sl = slice(512 * j, 512 * j + 512)
                    nc.sync.dma_start(out=qaug[0:64, h, sl],
                                      in_=qp_sb[64 * hh:64 * hh + 64, pr, sl])
                    nc.sync.dma_start(out=kaug[0:64, h, sl],
                                      in_=kp_sb[64 * hh:64 * hh + 64, pr, sl])

            def emit_vproj(st):
                pv = ps.tile([P, OC], f32, tag="proj", bufs=2)
                for k in range(NK):
                    nc.tensor.matmul(
                        pv[:, :], xt_sb[:, k, P * st:P * st + P], wv_sb[:, k, :],
                        start=(k == 0), stop=(k == NK - 1))
                pv4 = pv[:, :].rearrange("p (g x d) -> p g x d", g=2, x=2)
                nc.vector.tensor_copy(vv0[:, st, :, 0:64], pv4[:, :, 0, :])
                nc.vector.tensor_copy(vv1[:, st, :, 64:P], pv4[:, :, 1, :])

            def emit_pass1(pr, t):
                # scores [sq, sk] for one sq-tile, 2-head tile-packed; fused
                # (+causal mask) -> rowmax into m2 columns.
                ncols = (t + 1) * P
                nch = (ncols + 511) // 512
                m4a = tmp.tile([P, 4], f32, tag="m4a")
                m4b = tmp.tile([P, 4], f32, tag="m4b")
                for c in range(nch):
                    n = min(512, ncols - 512 * c)
                    sa = ps.tile([P, 512], f32, tag="s1", bufs=2)
                    sb_ = ps.tile([P, 512], f32, tag="s1", bufs=2)
                    last = c == nch - 1
                    nc.tensor.matmul(
                        sa[:, :n], qp_sb[0:64, pr, P * t:P * t + P],
                        kp_sb[0:64, pr, 512 * c:512 * c + n],
                        start=True, stop=not last, tile_position=(0, 0))
                    nc.tensor.matmul(
                        sb_[:, :n], qp_sb[64:P, pr, P * t:P * t + P],
                        kp_sb[64:P, pr, 512 * c:512 * c + n],
                        start=True, stop=not last, tile_position=(64, 0))
                    if last:
                        doff = n - P
                        nc.tensor.matmul(sa[:, doff:doff + P], ident16[:, :],
                                         cmask16[:, :], start=False, stop=True)
                        nc.tensor.matmul(sb_[:, doff:doff + P], ident16[:, :],
                                         cmask16[:, :], start=False, stop=True)
                    nc.vector.reduce_max(m4a[:, c:c + 1], sa[:, :n], axis=AX)
                    nc.vector.reduce_max(m4b[:, c:c + 1], sb_[:, :n], axis=AX)
                nc.vector.reduce_max(m2[:, pr, t:t + 1], m4a[:, 0:nch], axis=AX)
                nc.vector.reduce_max(m2[:, pr, 16 + t:16 + t + 1], m4b[:, 0:nch],
                                     axis=AX)

            def emit_mrow(pr, j):
                # -M for chunk j's four sq-tiles -> row 64 of qaug, per head.
                for hh in range(2):
                    mt_ps = ps.tile([4, P], f32, tag="s1", bufs=2)
                    nc.tensor.transpose(
                        mt_ps[:, :], m2[:, pr, 16 * hh + 4 * j:16 * hh + 4 * j + 4],
                        ident[:, :])
                    mt_t = tmp.tile([4, P], f16, tag="mt")
                    nc.vector.tensor_scalar_mul(mt_t[:, :], mt_ps[:, :], -1.0)
                    nc.sync.dma_start(
                        out=qaug[64:65, 2 * pr + hh,
                                 512 * j:512 * j + 512].rearrange(
                                     "q (t p) -> q t p", t=4),
                        in_=mt_t[:, :])

            def emit_pass2(h, j):
                # scoresT with folded -M, exp, causal zeroing, ctx+rowsum
                # accumulation, and normalization into ctxn.
                pr, hh = divmod(h, 2)
                ctxp = ps.tile([P, 512], f32, tag="ctx", bufs=2)
                nt_here = 4 * j + 4
                for t in range(nt_here):
                    if t < 4 * j:
                        qoff, n = 512 * j, 512
                    else:
                        qoff = P * t
                        n = 512 * j + 512 - P * t
                    s2p = ps.tile([P, 512], f32, tag="s2", bufs=2)
                    nc.tensor.matmul(
                        s2p[:, :n], kaug[:, h, P * t:P * t + P],
                        qaug[:, h, qoff:qoff + n], start=True, stop=True)
                    pt = strm.tile([P, 512], f16, tag="pt", bufs=6)
                    nc.scalar.activation(pt[:, :n], s2p[:, :n],
                                         mybir.ActivationFunctionType.Exp,
                                         scale=8.0)
                    if t >= 4 * j:
                        # zero strictly-upper block at the diagonal
                        nc.gpsimd.affine_select(
                            out=pt[:, 0:P], in_=pt[:, 0:P],
                            compare_op=ALU.is_ge, fill=0.0, base=0,
                            pattern=[[1, P]], channel_multiplier=-1)
                    lhsT = vv0[:, t, pr, :] if hh == 0 else vv1[:, t, pr, :]
                    nc.tensor.matmul(
                        ctxp[0:(65 if hh == 0 else P),
                             qoff - 512 * j:qoff - 512 * j + n],
                        lhsT, pt[:, :n],
                        start=(t == 0), stop=(t == nt_here - 1))
                # normalize: ctx / rowsum
                rsrow = 64 if hh == 0 else 0
                rr = tmp.tile([65, 512], f32, tag="rr")
                nc.vector.reciprocal(rr[rsrow:rsrow + 1, :],
                                     ctxp[rsrow:rsrow + 1, :])
                rb = tmp.tile([P, 512], f32, tag="rb")
                nc.sync.dma_start(
                    out=rb[64 * hh:64 * hh + 64, :],
                    in_=rr[rsrow:rsrow + 1, :].unsqueeze(1).broadcast_to(
                        (1, 64, 512)))
                nc.vector.tensor_mul(
                    ctxn[64 * hh:64 * hh + 64, pr, 512 * j:512 * j + 512],
                    ctxp[64 * hh:64 * hh + 64, :],
                    rb[64 * hh:64 * hh + 64, :])

            def emit_outproj(j):
                for oo in range(E // P):
                    po = ps.tile([P, 512], f32, tag="proj", bufs=2)
                    for kt in range(2):
                        nc.tensor.matmul(
                            po[:, :], wo_sb[:, kt, P * oo:P * oo + P],
                            ctxn[:, kt, 512 * j:512 * j + 512],
                            start=(kt == 0), stop=(kt == 1))
                    ob = strm.tile([P, 512], f32, tag="ob", bufs=3)
                    nc.scalar.copy(ob[:, :], po[:, :])
                    nc.sync.dma_start(out=outT_v[oo][:, 512 * j:512 * j + 512],
                                      in_=ob[:, :])

            # ---- interleaved attention pipeline ----
            for j in range(NJ):
                emit_aug(j)
                for st in range(4 * j, 4 * j + 4):
                    emit_vproj(st)
                for pr in range(2):
                    for t in range(4 * j, 4 * j + 4):
                        emit_pass1(pr, t)
                    emit_mrow(pr, j)
                    emit_pass2(2 * pr, j)
                    emit_pass2(2 * pr + 1, j)
                emit_outproj(j)

    nc.compile()
    return nc


def _get_nc():
    if "nc" not in _CACHE:
        _CACHE["nc"] = _build()
    return _CACHE["nc"]


def _make_cached_runner(nc):
    """Trace/compile the 8-core PJRT executable once; reuse on later calls.

    Mirrors concourse.bass2jax.run_bass_via_pjrt's multi-core branch, but
    keeps the jitted shard_map so repeat kernel() calls skip re-trace and
    re-lowering (the NEFF itself is already cached by neuronx_cc_hook).
    """
    import jax
    import jax.numpy as jnp
    from jax.sharding import Mesh, PartitionSpec
    from jax.experimental.shard_map import shard_map
    from concourse import bass2jax, mybir

    bass2jax.install_neuronx_cc_hook()
    partition_name = nc.partition_id_tensor.name if nc.partition_id_tensor else None
    in_names, out_names, out_avals = [], [], []
    for alloc in nc.m.functions[0].allocations:
        if not isinstance(alloc, mybir.MemoryLocationSet):
            continue
        name = alloc.memorylocations[0].name
        if alloc.kind == "ExternalInput":
            if name != partition_name:
                in_names.append(name)
        elif alloc.kind == "ExternalOutput":
            out_names.append(name)
            out_avals.append(jax.core.ShapedArray(
                tuple(alloc.tensor_shape), mybir.dt.np(alloc.dtype)))
    n_params = len(in_names)
    n_outs = len(out_avals)
    all_names = list(in_names) + list(out_names)
    if partition_name is not None:
        all_names.append(partition_name)

    def _body(*args):
        operands = list(args)
        if partition_name is not None:
            operands.append(bass2jax.partition_id_tensor())
        outs = bass2jax._bass_exec_p.bind(
            *operands,
            out_avals=tuple(out_avals),
            in_names=tuple(all_names),
            out_names=tuple(out_names),
            lowering_input_output_aliases=(),
            sim_require_finite=True,
            sim_require_nnan=True,
            nc=nc,
        )
        return tuple(outs)

    devices = jax.devices()[:NCORES]
    mesh = Mesh(np.asarray(devices), ("core",))
    in_specs = (PartitionSpec("core"),) * (n_params + n_outs)
    out_specs = (PartitionSpec("core"),) * n_outs
    sharded = jax.jit(
        shard_map(_body, mesh=mesh, in_specs=in_specs, out_specs=out_specs,
                  check_rep=False),
        donate_argnums=tuple(range(n_params, n_params + n_outs)),
        keep_unused=True)

    def run(in_maps):
        concat_in = [
            np.concatenate([np.asarray(in_maps[c][nm]) for c in range(NCORES)],
                           axis=0)
            for nm in in_names]
        concat_zeros = [
            np.zeros((NCORES * a.shape[0], *a.shape[1:]), a.dtype)
            for a in out_avals]
        out_arrs = sharded(*concat_in, *concat_zeros)
        return [
            {nm: np.asarray(out_arrs[i]).reshape(NCORES, *out_avals[i].shape)[c]
             for i, nm in enumerate(out_names)}
            for c in range(NCORES)]

    return run


def kernel(x, Wq, Wk, Wv, Wo):
    from concourse.bass_utils import run_bass_kernel_spmd

    # Force host numpy immediately: if the caller hands us jax arrays, any
    # .astype/.T on them would dispatch tiny jit programs to the neuron
    # backend, which wedges the device (known neuron-jit crash path).
    x, Wq, Wk, Wv, Wo = (np.asarray(a) for a in (x, Wq, Wk, Wv, Wo))

    nc = _get_nc()
    x16 = np.ascontiguousarray(x.astype(np.float16))
    Wq16 = Wq.astype(np.float16)
    Wk16 = Wk.astype(np.float16)
    Wv16 = Wv.astype(np.float16)
    Wo16 = Wo.astype(np.float16)

    xTs = [np.ascontiguousarray(x16[b].T) for b in range(B)]
    in_maps = []
    for c in range(NCORES):
        b, hg = divmod(c, HG)
        hsl = slice(OC * hg, OC * hg + OC)
        in_maps.append({
            "xt": xTs[b],
            "wq": np.ascontiguousarray(Wq16[hsl, :].T),
            "wk": np.ascontiguousarray(Wk16[hsl, :].T),
            "wv": np.ascontiguousarray(Wv16[hsl, :].T),
            "wo": np.ascontiguousarray(Wo16[:, hsl].T),
        })

    if "runner" in _CACHE:
        results = _CACHE["runner"](in_maps)
    else:
        # first call: compile + run through the sanctioned entry point,
        # then build the cached executable for subsequent calls
        results = run_bass_kernel_spmd(nc, in_maps, list(range(NCORES))).results
        _CACHE["runner"] = _make_cached_runner(nc)

    out = np.zeros((B, S, E), np.float32)
    for c in range(NCORES):
        b = c // HG
        out[b] += results[c]["outT"].T
    return out

